# revision 63
# baseline (speedup 1.0000x reference)
"""Trainium2 Bass kernel for nn_EntropyBottleneckLattice.

Math: the reference evaluates, for every (batch b, noise n, channel c),
p = d/dz sigmoid(L_c(z)) at z = x[b,c] + u[n,c], where L_c is a tiny
per-channel MLP tower (widths 1-3-3-3-3-1) with softplus-reparametrized
weights and tanh gating terms scaled by tanh(f_i); output is mean over n.

When all gate factors f_i == 0 (true for this problem's inputs), the tower
is affine per channel: L_c(z) = A_c * z + cc_c, so
    p = A_c * sigma'(s),  s = A_c*(x+u) + cc_c
    sigma'(s) = 0.25 * (1 - tanh(s/2)^2)
    lik[b,c]  = A_c/4 - (1/N) * sum_n (A_c/4) * tanh(s/2)^2

Device pipeline (per core, batch-sharded 512/8 = 64 rows; channel-major
layout, channels on partitions):
  - one DMA loads a host-packed fp16 blob: identity (PE weights),
    v1[c,b] = fp16(A x + cc - m_c), y1[c,n] = fp16(A u), plus A/4 and
    m_c/2 as fp32 bytes (single DMA -> single semaphore, so the 1-wait-slot
    matmul encodings never overflow)
  - main loop over 16 [128, 1024] PSUM chunks: s = v1 (+) y1 outer-sum via
    two identity-weight fp16 matmuls per 512 columns (stride-0 broadcast
    APs; fp32 PSUM accumulation is exact); t = tanh(0.5*s + m_c/2) on ACT
    (per-partition bias restores the channel mean); per-b DVE
    scalar_tensor_tensor (t * A/4) * t with fused accum_out giving
    G[c,b] = sum_n (A/4) t^2
  - final: lik_cb = -G/128 + A/4 (ACT affine), one DMA out channel-major;
    host transposes the 64KB result back to [b, c].

Sync-wait budget notes: fp32/fp32r matmuls (S3_LW), DVE TensorScalarPtr
(S2S2D2) and ACT (S3D3_AC) encodings accept only ONE semaphore wait, and
the kernel-tail SP drain only one as well. The kernel therefore (a) gives
every engine an early blob-touching op so the DMA semaphore is observed
once per engine, (b) pre-observes each psum slot's ACT release on the
previous chunk's last matmul via add_dep_helper, (c) never reuses t/dump
tiles (disjoint slices of one big tensor), and (d) funnels the tail drain
through per-engine SP nops.
"""

import os
from contextlib import ExitStack

import numpy as np

B, N, C = 512, 128, 256
NCORES = 8
B_SH = B // NCORES  # 64 batch rows per core
NBLK = C // 128  # channel blocks of 128 partitions

# blob column layout (fp16). v is centered per channel and stored as one
# fp16 part: v = fp16(v - m_c) + m_c, with m_c/2 applied later as the
# per-partition tanh bias (exact fp32 affine inside ACT). The centered
# residual spread is ~5x smaller than |v|, so one fp16 part keeps the
# common-mode error ~7e-5. y is a single fp16 part (|y| <= 0.06, rounding
# noise ~1e-5, independent across noise samples). The PE outer-sum
# s = v1+y1 accumulates exactly in fp32 PSUM (16-bit matmul path).
# a4 and m_c/2 (fp32) ride along as raw bytes, read via a bitcast view.
W_ID = 128
W_V = B_SH  # per block
W_Y = N  # per block
COL_ID = 0
COL_V = W_ID
COL_Y = COL_V + NBLK * W_V
COL_A4 = COL_Y + NBLK * W_Y  # must be even (fp32 bitcast view)
COL_MC = COL_A4 + 2 * NBLK
W_BLOB = COL_MC + 2 * NBLK  # 128 + 128 + 256 + 4 + 4 = 520 fp16 cols

_cache = {}


def _collapse_affine(inputs):
    """Per-channel affine collapse (float64): L_c(z) = A_c z + cc_c."""
    coef = np.ones((C, 1), dtype=np.float64)
    const = np.zeros((C, 1), dtype=np.float64)
    for i in range(5):
        m = inputs[f"m{i}"].astype(np.float64)
        H = np.log1p(np.exp(m))  # softplus
        b = inputs[f"b{i}"].astype(np.float64)[:, :, 0]
        coef = np.einsum("cij,cj->ci", H, coef)
        const = np.einsum("cij,cj->ci", H, const) + b
    return coef[:, 0], const[:, 0]


def _fp16_split(a):
    """Split fp32 array into two fp16 parts with a ~= p1 + p2 accurate to
    ~2^-24 relative."""
    a = np.ascontiguousarray(a, dtype=np.float32)
    p1 = a.astype(np.float16)
    p2 = (a - p1.astype(np.float32)).astype(np.float16)
    return p1, p2


def _build_fast_nc():
    """Build the Bass/Tile program for the f==0 fast path."""
    import concourse.bass as bass
    import concourse.tile as tile
    from concourse import mybir
    from concourse.tile_rust import add_dep_helper

    f32 = mybir.dt.float32
    f16 = mybir.dt.float16
    AF = mybir.ActivationFunctionType
    Alu = mybir.AluOpType

    _skip = set(os.environ.get("KERNEL_ABLATE", "").split(","))

    nc = bass.Bass("TRN2", target_bir_lowering=False, debug=False)

    blob_d = nc.dram_tensor("blob", [128, W_BLOB], f16, kind="ExternalInput").ap()
    o_d = nc.dram_tensor("out", [NBLK, 128, B_SH], f32, kind="ExternalOutput").ap()

    CHUNK = 1024  # psum columns per chunk = 8 b-groups of 128 noise cols
    BPC = CHUNK // N  # b values per chunk (8)
    NCHUNK = B_SH // BPC  # chunks per channel block (8)

    with tile.TileContext(nc) as tc, ExitStack() as ctx:
        consts = ctx.enter_context(tc.tile_pool(name="consts", bufs=1))
        mpsum = ctx.enter_context(tc.tile_pool(name="mpsum", bufs=4, space="PSUM"))

        blob = consts.tile([128, W_BLOB], f16, tag="blob")
        blob_dma = nc.gpsimd.dma_start(out=blob, in_=blob_d)

        ident_r = blob[:, COL_ID : COL_ID + 128]
        v = [
            blob[:, COL_V + k * W_V : COL_V + (k + 1) * W_V] for k in range(NBLK)
        ]
        y = [
            blob[:, COL_Y + k * W_Y : COL_Y + (k + 1) * W_Y] for k in range(NBLK)
        ]
        blob_f32 = blob.bitcast(f32)
        a4 = [
            blob_f32[:, COL_A4 // 2 + k : COL_A4 // 2 + k + 1] for k in range(NBLK)
        ]
        mc2 = [
            blob_f32[:, COL_MC // 2 + k : COL_MC // 2 + k + 1] for k in range(NBLK)
        ]

        G = consts.tile([128, NBLK * B_SH], f32, tag="G")

        # DVE and ACT observe the blob DMA once here; later ops on those
        # engines (1 sync-wait slot in their ISA encodings) then never need
        # the DMA wait themselves.
        scratch = consts.tile([128, 1], f32, tag="scratch")
        nc.vector.tensor_copy(scratch, a4[0])
        scratch2 = consts.tile([128, 1], f32, tag="scratch2")
        nc.scalar.copy(scratch2, a4[0])

        # One disjoint t-slice per chunk (no tile reuse): slot reuse would
        # create WAW/WAR waits that overflow the small per-instruction
        # sync-wait limits of the ACT/DVE ISA encodings.
        NCHUNK_ALL = NBLK * B_SH // BPC
        t_all = consts.tile([128, NCHUNK_ALL, CHUNK], f16, tag="t_all")

        PSUM_BUFS = 4
        tanh_insts = []  # per global chunk
        last_mm = last_stt = None
        g = 0

        # The PE clock ramp charges the first stretch of matmul instructions
        # at reduced p-states regardless of their size. Burn those slots
        # with N=1 dummy matmuls (~30ns each) into a throwaway psum slice so
        # the real 512-column matmuls start at the mid/full p-state
        # (30 dummies measured optimal: 41.9us -> 36.5us).
        if "mm" not in _skip:
            warm_ps = mpsum.tile([128, CHUNK], f32, tag="s")
            for _ in range(30):
                nc.tensor.matmul(
                    warm_ps[:, 0:1], ident_r, y[0][:, 0:1], start=True, stop=True
                )
        for k in range(NBLK):
            y_b = y[k].unsqueeze(1).broadcast_to([128, BPC, N])
            for ch in range(NCHUNK):
                ps = mpsum.tile([128, CHUNK], f32, tag="s")
                last_mm = None
                for j in range(CHUNK // 512):
                    b0 = ch * BPC + j * 4
                    v_b = (
                        v[k][:, b0 : b0 + 4].unsqueeze(2).broadcast_to([128, 4, N])
                    )
                    dst = ps[:, j * 512 : (j + 1) * 512]
                    if "mm" in _skip:
                        continue
                    nc.tensor.matmul(dst, ident_r, v_b, start=True, stop=False)
                    last_mm = nc.tensor.matmul(
                        dst, ident_r, y_b[:, 0:4, :], start=False, stop=True
                    )
                # fp32r matmuls (S3_LW) carry at most ONE sync wait. The
                # first matmul of chunk g+1 reuses the psum slot of chunk
                # g+1-PSUM_BUFS and would need both a PE WAW wait and an
                # ACT (tanh release) wait. Pre-observe the ACT release on
                # this chunk's last matmul (which has a free wait slot) so
                # the wrap matmul only needs the PE wait.
                if g >= PSUM_BUFS - 1 and last_mm is not None and tanh_insts:
                    add_dep_helper(
                        last_mm.ins,
                        tanh_insts[g - (PSUM_BUFS - 1)].ins,
                        sync=True,
                        reason="pre-observe psum release for next chunk",
                    )

                t_t = t_all[:, g, :]
                if "tanh" not in _skip:
                    th = nc.scalar.activation(
                        t_t, ps, AF.Tanh, bias=mc2[k], scale=0.5
                    )
                    tanh_insts.append(th)
                g += 1

                for bb in range(BPC):
                    if "stt" in _skip:
                        continue
                    b = ch * BPC + bb
                    tb = t_t[:, bb * N : (bb + 1) * N]
                    last_stt = nc.vector.scalar_tensor_tensor(
                        out=tb,  # in-place: each slice is read only by this op
                        in0=tb,
                        scalar=a4[k],
                        in1=tb,
                        op0=Alu.mult,
                        op1=Alu.mult,
                        accum_out=G[:, k * B_SH + b : k * B_SH + b + 1],
                    )

        # lik_cb = -G/128 + A/4, written channel-major; host transposes
        lik = consts.tile([128, NBLK, B_SH], f32, tag="lik")
        last_act = None
        for k in range(NBLK):
            last_act = nc.scalar.activation(
                lik[:, k, :],
                G[:, k * B_SH : (k + 1) * B_SH],
                AF.Identity,
                bias=a4[k],
                scale=-1.0 / N,
            )
        nc.gpsimd.dma_start(out=o_d.rearrange("k c b -> c k b"), in_=lik)

        # The kernel-tail drain (SP) gets a sync wait for every proc lane
        # the SP engine has not yet observed, but its ISA encoding holds
        # only a few. Funnel: SP nops each observe one lane (1 wait each),
        # so the final drain only needs the out-DMA lane.
        for tgt in (last_mm, last_act, last_stt, blob_dma):
            if tgt is None:
                continue
            nop = nc.sync.nop(nofuse=True, hint="tail_funnel")
            add_dep_helper(nop.ins, tgt.ins, sync=True, reason="tail funnel")

    return nc


def _run_fast(inputs, trace=False):
    from concourse.bass_utils import run_bass_kernel_spmd

    A, cc = _collapse_affine(inputs)
    x = inputs["inputs"].astype(np.float64)
    u = inputs["noise"].astype(np.float64)
    v_full = (A[None, :] * x + cc[None, :]).astype(np.float32)  # [B, C]
    y_full = (A[None, :] * u).astype(np.float32)  # [N, C]

    ident = np.eye(128, dtype=np.float32)
    a4 = (A / 4.0).astype(np.float32).reshape(NBLK, 128)

    y16 = np.ascontiguousarray(y_full, dtype=np.float16)
    in_maps = []
    for i in range(NCORES):
        blob = np.zeros((128, W_BLOB), dtype=np.float16)
        blob[:, COL_ID : COL_ID + 128] = ident.astype(np.float16)
        vs = v_full[i * B_SH : (i + 1) * B_SH].astype(np.float64)  # [B_SH, C]
        m_c = vs.mean(axis=0)  # [C]
        v1 = (vs - m_c[None, :]).astype(np.float16)  # centered, one fp16 part
        for k in range(NBLK):
            ck = slice(k * 128, (k + 1) * 128)
            blob[:, COL_V + k * W_V : COL_V + (k + 1) * W_V] = v1[:, ck].T
            blob[:, COL_Y + k * W_Y : COL_Y + (k + 1) * W_Y] = y16[:, ck].T
        # a4 and m_c/2 ride along as raw fp32 bytes viewed as fp16 pairs
        a4_bytes = np.stack([a4[k] for k in range(NBLK)], axis=1)  # [128, NBLK] f32
        blob[:, COL_A4 : COL_A4 + 2 * NBLK] = np.ascontiguousarray(
            a4_bytes, dtype=np.float32
        ).view(np.float16)
        mc2_bytes = np.ascontiguousarray(
            (m_c / 2.0).reshape(NBLK, 128).T, dtype=np.float32
        )  # [128, NBLK]
        blob[:, COL_MC : COL_MC + 2 * NBLK] = mc2_bytes.view(np.float16)
        in_maps.append({"blob": blob})

    if "nc" not in _cache:
        _cache["nc"] = _build_fast_nc()
    nc = _cache["nc"]

    res = run_bass_kernel_spmd(nc, in_maps, core_ids=list(range(NCORES)), trace=trace)
    _cache["last_results"] = res
    out = np.empty((B, C), dtype=np.float32)
    for i, r in enumerate(res.results):
        o = r["out"]  # [NBLK, 128, B_SH]
        for k in range(NBLK):
            out[i * B_SH : (i + 1) * B_SH, k * 128 : (k + 1) * 128] = o[k].T
    return out


def _run_general(inputs):
    """Fallback for nonzero gate factors: exact forward-mode evaluation on host."""
    x = inputs["inputs"].astype(np.float64)
    u = inputs["noise"].astype(np.float64)
    H = [np.log1p(np.exp(inputs[f"m{i}"].astype(np.float64))) for i in range(5)]
    bs = [inputs[f"b{i}"].astype(np.float64)[:, :, 0] for i in range(5)]
    tf = [np.tanh(inputs[f"f{i}"].astype(np.float64)[:, :, 0]) for i in range(4)]

    out = np.empty((B, C), dtype=np.float32)
    chunk = 32
    for s0 in range(0, B, chunk):
        s1 = min(s0 + chunk, B)
        z = x[s0:s1, None, :] + u[None, :, :]  # (bs, N, C)
        l = z[..., None]  # (bs, N, C, 1)
        d = np.ones_like(l)
        for i in range(5):
            l = np.einsum("cij,bncj->bnci", H[i], l) + bs[i]
            d = np.einsum("cij,bncj->bnci", H[i], d)
            if i < 4:
                t = np.tanh(l)
                l = l + tf[i] * t
                d = d * (1.0 + tf[i] * (1.0 - t * t))
        sig = 1.0 / (1.0 + np.exp(-l[..., 0]))
        p = sig * (1.0 - sig) * d[..., 0]  # (bs, N, C)
        out[s0:s1] = p.mean(axis=1).astype(np.float32)
    return out


def kernel(**inputs):
    inputs = {k: np.asarray(v) for k, v in inputs.items()}
    fast_ok = all(np.all(inputs[f"f{i}"] == 0) for i in range(4))
    if fast_ok:
        return _run_fast(inputs, trace=bool(int(os.environ.get("KERNEL_TRACE", "0"))))
    return _run_general(inputs)


# revision 68
# speedup vs baseline: 1.1186x; 1.1186x over previous
"""Trainium2 Bass kernel for nn_EntropyBottleneckLattice.

Math: the reference evaluates, for every (batch b, noise n, channel c),
p = d/dz sigmoid(L_c(z)) at z = x[b,c] + u[n,c], where L_c is a tiny
per-channel MLP tower (widths 1-3-3-3-3-1) with softplus-reparametrized
weights and tanh gating terms scaled by tanh(f_i); output is mean over n.

When all gate factors f_i == 0 (true for this problem's inputs), the tower
is affine per channel: L_c(z) = A_c * z + cc_c, so
    p = A_c * sigma'(s),  s = A_c*(x+u) + cc_c
    sigma'(s) = 0.25 * (1 - tanh(s/2)^2)
    lik[b,c]  = A_c/4 - (1/N) * sum_n (A_c/4) * tanh(s/2)^2

Device pipeline (per core, batch-sharded 512/8 = 64 rows; channel-major
layout, channels on partitions):
  - one DMA loads a host-packed fp16 blob: identity (PE weights),
    v1[c,b] = fp16(A x + cc - m_c), y1[c,n] = fp16(A u), plus A/4 and
    m_c/2 as fp32 bytes (single DMA -> single semaphore, so the 1-wait-slot
    matmul encodings never overflow)
  - main loop over 16 [128, 1024] PSUM chunks: s = v1 (+) y1 outer-sum via
    two identity-weight fp16 matmuls per 512 columns (stride-0 broadcast
    APs; fp32 PSUM accumulation is exact); t = tanh(0.5*s + m_c/2) on ACT
    (per-partition bias restores the channel mean); per-b DVE
    scalar_tensor_tensor (t * A/4) * t with fused accum_out giving
    G[c,b] = sum_n (A/4) t^2
  - final: lik_cb = -G/128 + A/4 (ACT affine), one DMA out channel-major;
    host transposes the 64KB result back to [b, c].

Sync-wait budget notes: fp32/fp32r matmuls (S3_LW), DVE TensorScalarPtr
(S2S2D2) and ACT (S3D3_AC) encodings accept only ONE semaphore wait, and
the kernel-tail SP drain only one as well. The kernel therefore (a) gives
every engine an early blob-touching op so the DMA semaphore is observed
once per engine, (b) pre-observes each psum slot's ACT release on the
previous chunk's last matmul via add_dep_helper, (c) never reuses t/dump
tiles (disjoint slices of one big tensor), and (d) funnels the tail drain
through per-engine SP nops.
"""

import os
from contextlib import ExitStack

import numpy as np

B, N, C = 512, 128, 256
NCORES = 8
B_SH = B // NCORES  # 64 batch rows per core
NBLK = C // 128  # channel blocks of 128 partitions

# blob column layout (fp16). v is centered per channel and stored as one
# fp16 part: v = fp16(v - m_c) + m_c, with m_c/2 applied later as the
# per-partition tanh bias (exact fp32 affine inside ACT). The centered
# residual spread is ~5x smaller than |v|, so one fp16 part keeps the
# common-mode error ~7e-5. y is a single fp16 part (|y| <= 0.06, rounding
# noise ~1e-5, independent across noise samples). The PE outer-sum
# s = v1+y1 accumulates exactly in fp32 PSUM (16-bit matmul path).
# a4 and m_c/2 (fp32) ride along as raw bytes, read via a bitcast view.
W_ID = 128
W_V = B_SH  # per block
W_Y = N  # per block
COL_ID = 0
COL_V = W_ID
COL_Y = COL_V + NBLK * W_V
COL_A4 = COL_Y + NBLK * W_Y  # must be even (fp32 bitcast view)
COL_MC = COL_A4 + 2 * NBLK
W_BLOB = COL_MC + 2 * NBLK  # 128 + 128 + 256 + 4 + 4 = 520 fp16 cols

_cache = {}


def _collapse_affine(inputs):
    """Per-channel affine collapse (float64): L_c(z) = A_c z + cc_c."""
    coef = np.ones((C, 1), dtype=np.float64)
    const = np.zeros((C, 1), dtype=np.float64)
    for i in range(5):
        m = inputs[f"m{i}"].astype(np.float64)
        H = np.log1p(np.exp(m))  # softplus
        b = inputs[f"b{i}"].astype(np.float64)[:, :, 0]
        coef = np.einsum("cij,cj->ci", H, coef)
        const = np.einsum("cij,cj->ci", H, const) + b
    return coef[:, 0], const[:, 0]


def _fp16_split(a):
    """Split fp32 array into two fp16 parts with a ~= p1 + p2 accurate to
    ~2^-24 relative."""
    a = np.ascontiguousarray(a, dtype=np.float32)
    p1 = a.astype(np.float16)
    p2 = (a - p1.astype(np.float32)).astype(np.float16)
    return p1, p2


def _build_fast_nc():
    """Build the Bass/Tile program for the f==0 fast path."""
    import concourse.bass as bass
    import concourse.tile as tile
    from concourse import mybir
    from concourse.tile_rust import add_dep_helper

    f32 = mybir.dt.float32
    f16 = mybir.dt.float16
    AF = mybir.ActivationFunctionType
    Alu = mybir.AluOpType

    _skip = set(os.environ.get("KERNEL_ABLATE", "").split(","))

    nc = bass.Bass("TRN2", target_bir_lowering=False, debug=False)

    blob_d = nc.dram_tensor("blob", [128, W_BLOB], f16, kind="ExternalInput").ap()
    o_d = nc.dram_tensor("out", [NBLK, 128, B_SH], f32, kind="ExternalOutput").ap()

    CHUNK = 1024  # psum columns per chunk = 8 b-groups of 128 noise cols
    BPC = CHUNK // N  # b values per chunk (8)
    NCHUNK = B_SH // BPC  # chunks per channel block (8)

    with tile.TileContext(nc) as tc, ExitStack() as ctx:
        consts = ctx.enter_context(tc.tile_pool(name="consts", bufs=1))
        mpsum = ctx.enter_context(tc.tile_pool(name="mpsum", bufs=4, space="PSUM"))

        blob = consts.tile([128, W_BLOB], f16, tag="blob")
        blob_dma = nc.gpsimd.dma_start(out=blob, in_=blob_d)

        ident_r = blob[:, COL_ID : COL_ID + 128]
        v = [
            blob[:, COL_V + k * W_V : COL_V + (k + 1) * W_V] for k in range(NBLK)
        ]
        y = [
            blob[:, COL_Y + k * W_Y : COL_Y + (k + 1) * W_Y] for k in range(NBLK)
        ]
        blob_f32 = blob.bitcast(f32)
        a4 = [
            blob_f32[:, COL_A4 // 2 + k : COL_A4 // 2 + k + 1] for k in range(NBLK)
        ]
        mc2 = [
            blob_f32[:, COL_MC // 2 + k : COL_MC // 2 + k + 1] for k in range(NBLK)
        ]

        G = consts.tile([128, NBLK * B_SH], f32, tag="G")

        # DVE and ACT observe the blob DMA once here; later ops on those
        # engines (1 sync-wait slot in their ISA encodings) then never need
        # the DMA wait themselves.
        scratch = consts.tile([128, 1], f32, tag="scratch")
        nc.vector.tensor_copy(scratch, a4[0])
        scratch2 = consts.tile([128, 1], f32, tag="scratch2")
        nc.scalar.copy(scratch2, a4[0])
        scratch4 = consts.tile([128, 1], f32, tag="scratch4")
        nc.gpsimd.tensor_copy(scratch4, a4[0])

        # One disjoint t-slice per chunk (no tile reuse): slot reuse would
        # create WAW/WAR waits that overflow the small per-instruction
        # sync-wait limits of the ACT/DVE ISA encodings.
        NCHUNK_ALL = NBLK * B_SH // BPC
        t_all = consts.tile([128, NCHUNK_ALL, CHUNK], f16, tag="t_all")

        PSUM_BUFS = 4
        tanh_insts = []  # per global chunk
        last_mm = last_stt = last_ptt = None
        g = 0

        # The PE clock ramp charges the first stretch of matmul instructions
        # at reduced p-states regardless of their size. Burn those slots
        # with N=1 dummy matmuls (~30ns each) into a throwaway psum slice so
        # the real 512-column matmuls start at the mid/full p-state
        # (30 dummies measured optimal: 41.9us -> 36.5us).
        if "mm" not in _skip:
            warm_ps = mpsum.tile([128, CHUNK], f32, tag="s")
            for _ in range(30):
                nc.tensor.matmul(
                    warm_ps[:, 0:1], ident_r, y[0][:, 0:1], start=True, stop=True
                )
        for k in range(NBLK):
            y_b = y[k].unsqueeze(1).broadcast_to([128, BPC, N])
            for ch in range(NCHUNK):
                ps = mpsum.tile([128, CHUNK], f32, tag="s")
                last_mm = None
                for j in range(CHUNK // 512):
                    b0 = ch * BPC + j * 4
                    v_b = (
                        v[k][:, b0 : b0 + 4].unsqueeze(2).broadcast_to([128, 4, N])
                    )
                    dst = ps[:, j * 512 : (j + 1) * 512]
                    if "mm" in _skip:
                        continue
                    nc.tensor.matmul(dst, ident_r, v_b, start=True, stop=False)
                    last_mm = nc.tensor.matmul(
                        dst, ident_r, y_b[:, 0:4, :], start=False, stop=True
                    )
                # fp32r matmuls (S3_LW) carry at most ONE sync wait. The
                # first matmul of chunk g+1 reuses the psum slot of chunk
                # g+1-PSUM_BUFS and would need both a PE WAW wait and an
                # ACT (tanh release) wait. Pre-observe the ACT release on
                # this chunk's last matmul (which has a free wait slot) so
                # the wrap matmul only needs the PE wait.
                if g >= PSUM_BUFS - 1 and last_mm is not None and tanh_insts:
                    add_dep_helper(
                        last_mm.ins,
                        tanh_insts[g - (PSUM_BUFS - 1)].ins,
                        sync=True,
                        reason="pre-observe psum release for next chunk",
                    )

                t_t = t_all[:, g, :]
                if "tanh" not in _skip:
                    th = nc.scalar.activation(
                        t_t, ps, AF.Tanh, bias=mc2[k], scale=0.5
                    )
                    tanh_insts.append(th)
                g += 1

                for bb in range(BPC):
                    if "stt" in _skip:
                        continue
                    b = ch * BPC + bb
                    tb = t_t[:, bb * N : (bb + 1) * N]
                    acc = G[:, k * B_SH + b : k * B_SH + b + 1]
                    if bb < 5:
                        # DVE: fused (t * A/4) * t with accumulate
                        last_stt = nc.vector.scalar_tensor_tensor(
                            out=tb,  # in-place; slice not read again
                            in0=tb,
                            scalar=a4[k],
                            in1=tb,
                            op0=Alu.mult,
                            op1=Alu.mult,
                            accum_out=acc,
                        )
                    else:
                        # Offload the square to the otherwise-idle GPSIMD,
                        # then a cheap single-source DVE tensor_scalar does
                        # the scaled accumulate (gets the fp16 perf mode).
                        last_ptt = nc.gpsimd.tensor_tensor(
                            out=tb, in0=tb, in1=tb, op=Alu.mult
                        )
                        last_stt = nc.vector.tensor_scalar(
                            out=tb,
                            in0=tb,
                            scalar1=a4[k],
                            scalar2=0.0,
                            op0=Alu.mult,
                            op1=Alu.add,
                            accum_out=acc,
                        )

        # lik_cb = -G/128 + A/4, written channel-major; host transposes
        lik = consts.tile([128, NBLK, B_SH], f32, tag="lik")
        last_act = None
        for k in range(NBLK):
            last_act = nc.scalar.activation(
                lik[:, k, :],
                G[:, k * B_SH : (k + 1) * B_SH],
                AF.Identity,
                bias=a4[k],
                scale=-1.0 / N,
            )
        nc.gpsimd.dma_start(out=o_d.rearrange("k c b -> c k b"), in_=lik)

        # The kernel-tail drain (SP) gets a sync wait for every proc lane
        # the SP engine has not yet observed, but its ISA encoding holds
        # only a few. Funnel: SP nops each observe one lane (1 wait each),
        # so the final drain only needs the out-DMA lane.
        for tgt in (last_mm, last_act, last_stt, last_ptt, blob_dma):
            if tgt is None:
                continue
            nop = nc.sync.nop(nofuse=True, hint="tail_funnel")
            add_dep_helper(nop.ins, tgt.ins, sync=True, reason="tail funnel")

    return nc


def _run_fast(inputs, trace=False):
    from concourse.bass_utils import run_bass_kernel_spmd

    A, cc = _collapse_affine(inputs)
    x = inputs["inputs"].astype(np.float64)
    u = inputs["noise"].astype(np.float64)
    v_full = (A[None, :] * x + cc[None, :]).astype(np.float32)  # [B, C]
    y_full = (A[None, :] * u).astype(np.float32)  # [N, C]

    ident = np.eye(128, dtype=np.float32)
    a4 = (A / 4.0).astype(np.float32).reshape(NBLK, 128)

    y16 = np.ascontiguousarray(y_full, dtype=np.float16)
    in_maps = []
    for i in range(NCORES):
        blob = np.zeros((128, W_BLOB), dtype=np.float16)
        blob[:, COL_ID : COL_ID + 128] = ident.astype(np.float16)
        vs = v_full[i * B_SH : (i + 1) * B_SH].astype(np.float64)  # [B_SH, C]
        m_c = vs.mean(axis=0)  # [C]
        v1 = (vs - m_c[None, :]).astype(np.float16)  # centered, one fp16 part
        for k in range(NBLK):
            ck = slice(k * 128, (k + 1) * 128)
            blob[:, COL_V + k * W_V : COL_V + (k + 1) * W_V] = v1[:, ck].T
            blob[:, COL_Y + k * W_Y : COL_Y + (k + 1) * W_Y] = y16[:, ck].T
        # a4 and m_c/2 ride along as raw fp32 bytes viewed as fp16 pairs
        a4_bytes = np.stack([a4[k] for k in range(NBLK)], axis=1)  # [128, NBLK] f32
        blob[:, COL_A4 : COL_A4 + 2 * NBLK] = np.ascontiguousarray(
            a4_bytes, dtype=np.float32
        ).view(np.float16)
        mc2_bytes = np.ascontiguousarray(
            (m_c / 2.0).reshape(NBLK, 128).T, dtype=np.float32
        )  # [128, NBLK]
        blob[:, COL_MC : COL_MC + 2 * NBLK] = mc2_bytes.view(np.float16)
        in_maps.append({"blob": blob})

    if "nc" not in _cache:
        _cache["nc"] = _build_fast_nc()
    nc = _cache["nc"]

    res = run_bass_kernel_spmd(nc, in_maps, core_ids=list(range(NCORES)), trace=trace)
    _cache["last_results"] = res
    out = np.empty((B, C), dtype=np.float32)
    for i, r in enumerate(res.results):
        o = r["out"]  # [NBLK, 128, B_SH]
        for k in range(NBLK):
            out[i * B_SH : (i + 1) * B_SH, k * 128 : (k + 1) * 128] = o[k].T
    return out


def _run_general(inputs):
    """Fallback for nonzero gate factors: exact forward-mode evaluation on host."""
    x = inputs["inputs"].astype(np.float64)
    u = inputs["noise"].astype(np.float64)
    H = [np.log1p(np.exp(inputs[f"m{i}"].astype(np.float64))) for i in range(5)]
    bs = [inputs[f"b{i}"].astype(np.float64)[:, :, 0] for i in range(5)]
    tf = [np.tanh(inputs[f"f{i}"].astype(np.float64)[:, :, 0]) for i in range(4)]

    out = np.empty((B, C), dtype=np.float32)
    chunk = 32
    for s0 in range(0, B, chunk):
        s1 = min(s0 + chunk, B)
        z = x[s0:s1, None, :] + u[None, :, :]  # (bs, N, C)
        l = z[..., None]  # (bs, N, C, 1)
        d = np.ones_like(l)
        for i in range(5):
            l = np.einsum("cij,bncj->bnci", H[i], l) + bs[i]
            d = np.einsum("cij,bncj->bnci", H[i], d)
            if i < 4:
                t = np.tanh(l)
                l = l + tf[i] * t
                d = d * (1.0 + tf[i] * (1.0 - t * t))
        sig = 1.0 / (1.0 + np.exp(-l[..., 0]))
        p = sig * (1.0 - sig) * d[..., 0]  # (bs, N, C)
        out[s0:s1] = p.mean(axis=1).astype(np.float32)
    return out


def kernel(**inputs):
    inputs = {k: np.asarray(v) for k, v in inputs.items()}
    fast_ok = all(np.all(inputs[f"f{i}"] == 0) for i in range(4))
    if fast_ok:
        return _run_fast(inputs, trace=bool(int(os.environ.get("KERNEL_TRACE", "0"))))
    return _run_general(inputs)
